# revision 24
# baseline (speedup 1.0000x reference)
"""Trainium2 Bass kernel for AttentionNet:
out[bh,l,m] = sum_d w3[d] * tanh((X@W1.T+b1)[bh,l,d] * (Y@W2.T+b2)[bh,m,d]) + b3

Sharding: data-parallel over the fused B*H axis. 32 bh-slices / 8 cores =
4 bh per core (core c gets batch b=c, all 4 heads). Params replicated.

Per-core pipeline (fully unrolled, Tile framework handles all sync). All
heavy tensors live in the (d x free) layout with the hidden dim d on the
128 SBUF partitions, so the final d-contraction can run on the PE:

  - linear heads: DMA X[bh] natural -> PE transpose (identity matmul) ->
    DVE copy PSUM->SBUF -> PE matmul with host-pre-transposed W1 ->
    DVE tensor_scalar_add drain (bias add + cast to fp16) = XpT/YpT
    (d x 128, fp16).
  - expand: YpT replicated G=16x along the free dim (one DVE stride-0
    broadcast copy, ~1.4us) so the product can run as big stride-1
    tensor_tensor ops (fp16 2x_1P DVE mode; per-partition-scalar
    tensor_scalar would be 1x and ~3x slower).
  - product: 8 DVE tensor_tensor instrs per bh, each FD=2048 covering
    (all 128 m) x (16 l): prod[d, m*128+l] = XpT[d,l]*YpT[d,m].
  - tanh: 2 ScalarE instructions per bh (FD=8192, fp16). This is the
    bottleneck engine: ~1.2-1.5 elem/cycle/lane, ~45-58us/core total.
    ScalarE must stay tanh-only: mixing activation functions from
    different table-sets costs ~2.7us per table reload.
  - reduce: per m, PE matmul lhsT = tanh slice (d x 128 fp16, FWL) and
    rhs = w3 (d x 1): out column = psum[:, m], natural (l, m) layout;
    ~64ns per ldweights+matmul pair.
  - drain: DVE tensor_scalar_add(+b3) PSUM->SBUF, DMA out.

All 16-bit stages use fp16 (same engine rates as bf16, 3 more mantissa
bits: rel err 3.7e-4 vs 3.0e-3). Measured steady state ~55-75us/rep
depending on device power state (ACT-throughput-bound); other engines
(DVE ~46us, PE ~37us) hide under it.
"""

import numpy as np

B, H, L, D = 8, 4, 128, 128
NCORES = 8
BH_PER_CORE = (B * H) // NCORES  # 4
CHUNK_M = 64  # columns of the output per ACT instruction

_CACHE = {}


def _build(reps=1, chunk_m=CHUNK_M, bufs_big=3, skip_product=False, skip_act=False,
           skip_reduce=False, psum_bufs=2, tanh_chunks=2, io_bufs=3, lin_bufs=2,
           bias_on_act=False, act_read_const=False, m_split=False, use_fp16=True,
           dma_expand=False, xnt_on_act=False, l_split_act=False):
    import concourse.mybir as mybir
    from concourse import bacc
    from concourse._compat import get_trn_type
    from concourse.tile import TileContext

    f32 = mybir.dt.float32
    bf16 = mybir.dt.float16 if use_fp16 else mybir.dt.bfloat16
    TANH = mybir.ActivationFunctionType.Tanh

    nc = bacc.Bacc(get_trn_type() or "TRN2", target_bir_lowering=False, debug=False)

    Xd = nc.declare_dram_parameter("X", [BH_PER_CORE, L, D], f32, isOutput=False)
    Yd = nc.declare_dram_parameter("Y", [BH_PER_CORE, L, D], f32, isOutput=False)
    W1Td = nc.declare_dram_parameter("W1T", [D, D], f32, isOutput=False)
    W2Td = nc.declare_dram_parameter("W2T", [D, D], f32, isOutput=False)
    b1d = nc.declare_dram_parameter("b1c", [D, 1], f32, isOutput=False)
    b2d = nc.declare_dram_parameter("b2c", [D, 1], f32, isOutput=False)
    w3d = nc.declare_dram_parameter("w3c", [D, 1], bf16, isOutput=False)
    b3d = nc.declare_dram_parameter("b3c", [L, 1], f32, isOutput=False)
    identd = nc.declare_dram_parameter("ident", [L, L], f32, isOutput=False)
    Od = nc.declare_dram_parameter("out", [BH_PER_CORE, L, L], f32, isOutput=True)

    with TileContext(nc) as tc:
        with (
            tc.tile_pool(name="const", bufs=1) as cpool,
            tc.tile_pool(name="io", bufs=io_bufs) as iopool,
            tc.tile_pool(name="lin", bufs=lin_bufs) as linpool,
            tc.tile_pool(name="big", bufs=bufs_big) as bigpool,
            tc.tile_pool(name="ps_t", bufs=psum_bufs, space="PSUM") as pst,
            tc.tile_pool(name="ps_o", bufs=2, space="PSUM") as pso,
        ):
            w1t = cpool.tile([D, D], f32, tag="w1t")
            nc.sync.dma_start(w1t[:], W1Td[:])
            w2t = cpool.tile([D, D], f32, tag="w2t")
            nc.sync.dma_start(w2t[:], W2Td[:])
            b1c = cpool.tile([D, 1], f32, tag="b1c")
            nc.sync.dma_start(b1c[:], b1d[:])
            b2c = cpool.tile([D, 1], f32, tag="b2c")
            nc.sync.dma_start(b2c[:], b2d[:])
            w3c = cpool.tile([D, 1], bf16, tag="w3c")
            nc.sync.dma_start(w3c[:], w3d[:])
            b3c = cpool.tile([L, 1], f32, tag="b3c")
            nc.sync.dma_start(b3c[:], b3d[:])
            ident = cpool.tile([L, L], f32, tag="ident")
            nc.sync.dma_start(ident[:], identd[:])
            actsrc = None
            if act_read_const:
                actsrc = cpool.tile([D, L * L], bf16, tag="actsrc")
                nc.vector.tensor_copy(actsrc[:, 0:L], ident[:])

            G = 16  # l-block width per product instruction (FD = 128*G)
            for bh in [i % BH_PER_CORE for i in range(reps * BH_PER_CORE)]:
                pbf = {}
                for src, wt, bc, nm in (
                    (Xd, w1t, b1c, "x"),
                    (Yd, w2t, b2c, "y"),
                ):
                    xn = iopool.tile([L, D], f32, tag="xn")
                    nc.sync.dma_start(xn[:], src[bh])
                    tps = pst.tile([D, L], f32, tag="tps")
                    nc.tensor.transpose(tps[:], xn[:], ident[:])
                    xnt = linpool.tile([D, L], f32, tag="xnt")
                    if xnt_on_act:
                        nc.scalar.copy(xnt[:], tps[:])
                    else:
                        nc.vector.tensor_copy(xnt[:], tps[:])
                    lps = pst.tile([D, L], f32, tag="lps")
                    nc.tensor.matmul(lps[:], wt[:], xnt[:], start=True, stop=True)
                    t = linpool.tile([D, L], bf16, tag=nm + "bf")
                    if bias_on_act:
                        nc.scalar.activation(
                            t[:], lps[:], mybir.ActivationFunctionType.Identity,
                            bias=bc[:],
                        )
                    else:
                        nc.vector.tensor_scalar_add(t[:], lps[:], bc[:])
                    pbf[nm] = t

                # expand YpT 16x along free dim so the product can run as
                # large stride-1 tensor_tensor ops (2x bf16 DVE mode)
                yexp = linpool.tile([D, L * G], bf16, tag="yexp")
                if dma_expand:
                    nc.sync.dma_start(
                        yexp[:].rearrange("p (m g) -> p m g", g=G),
                        pbf["y"][:]
                        .rearrange("p (m a) -> p m a", a=1)
                        .broadcast_to([D, L, G]),
                    )
                else:
                    nc.vector.tensor_copy(
                        yexp[:].rearrange("p (m g) -> p m g", g=G),
                        pbf["y"][:]
                        .rearrange("p (m a) -> p m a", a=1)
                        .broadcast_to([D, L, G]),
                    )

                # prod[d, m*L + l] = XpT[d, l] * YpT[d, m]
                out_ps = pso.tile([L, L], f32, tag="ops")
                HM = L // tanh_chunks  # m-columns per tanh chunk
                yex3 = yexp[:].rearrange("p (m g) -> p m g", g=G)
                if not m_split:
                    prod = bigpool.tile([D, L * L], bf16, tag="prod")
                    prod3 = prod[:].rearrange("p (m l) -> p m l", l=L)
                    for b in range(L // G):
                        if skip_product and b > 0:
                            continue
                        in0 = (
                            pbf["x"][:, b * G : (b + 1) * G]
                            .rearrange("p (a g) -> p a g", a=1)
                            .broadcast_to([D, L, G])
                        )
                        nc.vector.tensor_tensor(
                            prod3[:, :, b * G : (b + 1) * G],
                            in0,
                            yex3,
                            op=mybir.AluOpType.mult,
                        )
                if l_split_act and not m_split and not skip_act:
                    # tanh sliced by l-halves: chunk h depends on only the
                    # first/last 4 product TTs instead of all 8
                    tanh_f = bigpool.tile([D, L * L], bf16, tag="tanhf")
                    tanh3 = tanh_f[:].rearrange("p (m l) -> p m l", l=L)
                    HL = L // tanh_chunks
                    for h in range(tanh_chunks):
                        nc.scalar.activation(
                            tanh3[:, :, h * HL : (h + 1) * HL],
                            prod3[:, :, h * HL : (h + 1) * HL],
                            TANH,
                        )
                    for m in range(L):
                        if skip_reduce and m > 0:
                            continue
                        nc.tensor.matmul(
                            out_ps[:, m : m + 1],
                            tanh_f[:, m * L : (m + 1) * L],
                            w3c[:],
                            start=True,
                            stop=True,
                        )
                    outs = iopool.tile([L, L], f32, tag="outs")
                    nc.vector.tensor_scalar_add(outs[:], out_ps[:], b3c[:])
                    nc.sync.dma_start(Od[bh], outs[:])
                    continue
                for half in range(tanh_chunks):
                    if m_split:
                        prod = bigpool.tile([D, HM * L], bf16, tag="prod")
                        prod3 = prod[:].rearrange("p (m l) -> p m l", l=L)
                        for b in range(L // G):
                            if skip_product and b > 0:
                                continue
                            in0 = (
                                pbf["x"][:, b * G : (b + 1) * G]
                                .rearrange("p (a g) -> p a g", a=1)
                                .broadcast_to([D, HM, G])
                            )
                            nc.vector.tensor_tensor(
                                prod3[:, :, b * G : (b + 1) * G],
                                in0,
                                yex3[:, half * HM : (half + 1) * HM, :],
                                op=mybir.AluOpType.mult,
                            )
                        pr_off = 0
                    else:
                        pr_off = half * HM * L
                    if skip_act:
                        tanh_t = prod
                        tslice = lambda j: tanh_t[:, pr_off + j * L : pr_off + (j + 1) * L]
                    else:
                        tanh_t = bigpool.tile([D, HM * L], bf16, tag="tanh")
                        asrc = actsrc if act_read_const else prod
                        aoff = 0 if act_read_const else pr_off
                        nc.scalar.activation(
                            tanh_t[:], asrc[:, aoff : aoff + HM * L], TANH
                        )
                        tslice = lambda j: tanh_t[:, j * L : (j + 1) * L]
                    for j in range(HM):
                        if skip_reduce and j > 0:
                            continue
                        m = half * HM + j
                        nc.tensor.matmul(
                            out_ps[:, m : m + 1],
                            tslice(j),
                            w3c[:],
                            start=True,
                            stop=True,
                        )
                outs = iopool.tile([L, L], f32, tag="outs")
                nc.vector.tensor_scalar_add(outs[:], out_ps[:], b3c[:])
                nc.sync.dma_start(Od[bh], outs[:])

    nc.compile()
    return nc


def _get_nc(reps=1, **kwargs):
    key = ("nc", reps, tuple(sorted(kwargs.items())))
    if key not in _CACHE:
        _CACHE[key] = _build(reps, **kwargs)
    return _CACHE[key]


def _make_in_maps(X, Y, W1, b1, W2, b2, w3, b3):
    X = np.ascontiguousarray(np.asarray(X, dtype=np.float32)).reshape(B * H, L, D)
    Y = np.ascontiguousarray(np.asarray(Y, dtype=np.float32)).reshape(B * H, L, D)
    W1T = np.ascontiguousarray(np.asarray(W1, dtype=np.float32).T)
    W2T = np.ascontiguousarray(np.asarray(W2, dtype=np.float32).T)
    b1c = np.ascontiguousarray(np.asarray(b1, dtype=np.float32).reshape(D, 1))
    b2c = np.ascontiguousarray(np.asarray(b2, dtype=np.float32).reshape(D, 1))
    w3c = np.asarray(w3, dtype=np.float32).astype(np.float16).reshape(D, 1)
    b3c = np.full((L, 1), float(np.asarray(b3)), dtype=np.float32)
    ident = np.eye(L, dtype=np.float32)
    in_maps = []
    for c in range(NCORES):
        sl = slice(c * BH_PER_CORE, (c + 1) * BH_PER_CORE)
        in_maps.append(
            {
                "X": np.ascontiguousarray(X[sl]),
                "Y": np.ascontiguousarray(Y[sl]),
                "W1T": W1T,
                "W2T": W2T,
                "b1c": b1c,
                "b2c": b2c,
                "w3c": w3c,
                "b3c": b3c,
                "ident": ident,
            }
        )
    return in_maps


def _run(in_maps, trace=False, **kwargs):
    from concourse.bass_utils import run_bass_kernel_spmd

    nc = _get_nc()
    return run_bass_kernel_spmd(
        nc, in_maps, core_ids=list(range(NCORES)), trace=trace, **kwargs
    )


def kernel(X, Y, W1, b1, W2, b2, w3, b3):
    in_maps = _make_in_maps(X, Y, W1, b1, W2, b2, w3, b3)
    last_err = None
    for sleep_s in (0, 5, 20, 45):
        try:
            if sleep_s:
                import time

                time.sleep(sleep_s)
            res = _run(in_maps, trace=False)
            break
        except Exception as e:  # sporadic device-unrecoverable; retry
            last_err = e
    else:
        raise last_err
    out = np.stack([np.asarray(res.results[c]["out"]) for c in range(NCORES)])
    return out.reshape(B, H, L, L)
